# revision 54
# baseline (speedup 1.0000x reference)
"""Trainium2 Bass kernel for a 5-layer bidirectional GRU (T=256, B=128, I=128, H=512, O=1).

Strategy (direction-parallel, dense PE):
  - 8 cores = 2 direction groups x 4 batch shards (BC=32).
    Cores 0-3 run the forward chains, cores 4-7 the backward chains on
    host-time-reversed input (identical SPMD program; direction is data).
  - Per-block pairwise AllGathers (groups [c, c+4]) exchange each core's
    time-reversed activations while compute continues; the next layer's
    input is [own acts (local, natural), partner acts (gathered, already
    in own time order)]. Partner half selected branch-free via
    (g0-g1)*sel + g1 with a per-core sel scalar.
  - Recurrent matmul: weight-stationary fp8 Whh (x4096 host scale — max
    |w| stays under the e4m3 240 ceiling) over 48 [128x128] tiles per
    step, split z/r/n into three PSUM banks so the gate sigmoids start
    mid-sweep; gx pre-accumulated into the z/r PSUM via identity matmuls
    (doubling as the bank clear); b_hn via one K=4 matmul against an
    indicator rhs; fp8 descale folded into the activation scales.
  - Input-side gx matmuls run on a half-block-ahead conveyor (~6 mms
    due each step) so the PE stays dense through the elementwise window
    (keeps HAM at full clock).
  - Step-major SBUF layouts keep every elementwise op contiguous
    [128,128] bf16 (DVE 2x mode); elem chain: sigmoid(z), 1-z via
    sigmoid(-x), z*h_prev overlapping tanh, then 2 muls + add.
"""

import sys

sys.path.insert(0, "/opt/trn_rl_repo")

import numpy as np
import ml_dtypes

import concourse.bass as bass
from concourse.bass import _add_dep_helper
import concourse.bacc as bacc
import concourse.mybir as mybir
import concourse.tile as tile
from concourse.vector_clock import ScopedClock, VectorClock
from concourse.bass_utils import run_bass_kernel_spmd

BF16 = mybir.dt.bfloat16
F32 = mybir.dt.float32
FP8 = mybir.dt.float8e4
AF = mybir.ActivationFunctionType
OP = mybir.AluOpType

T, B, I, H, O, L = 256, 128, 128, 512, 1, 5
G3 = 3 * H              # 1536
NCORES = 8
NSHARD = 4              # batch shards per direction group
BC = B // NSHARD        # 32 batch per core
TOK = T * BC            # 8192 token columns per core
NBLK = T // 32          # 8 blocks of 32 timesteps
BLKC = 32 * BC          # 1024 token columns per block
HB = BLKC // 2          # 512 token columns per half block
KH = H // 128           # 4 k-chunks of the hidden dim
M3 = G3 // 128          # 12 m-chunks of the gate dim
P = 128

WHH_FP8 = True
WSCALE = 4096.0 if WHH_FP8 else 1.0
WHH_DT = FP8 if WHH_FP8 else BF16
PER_BLOCK_CC = True    # one AllGather per block vs one per layer
CONVEYOR = True        # fine-grained gx interleave vs block-granular
ORDER_DEPS = False      # pin gx units into each step's elem window on PE


class ChunkedDrainTC(tile.TileContext):
    """Work around walrus's 2-sync-wait limit on the kernel-tail drain by
    splitting the final drain into several drains with <=2 waits each."""

    def _drain_and_barrier(self, tick_clock, wait_clock):
        gc = tick_clock.global_clock
        n = len(gc)
        for i0 in range(0, n, 2):
            vec = [0] * n
            any_set = False
            for i in range(i0, min(i0 + 2, n)):
                vec[i] = gc[i]
                any_set = any_set or gc[i] > 0
            if not any_set:
                continue
            di = self.nc.sync.drain()
            wait_clock.add_sem_waits(di.ins, ScopedClock({None: VectorClock(vec)}))
        self.nc.all_engine_barrier()
        popped = self.nc._tile_sem_poison_stack.pop()
        assert popped is self._sem_poison
        self.nc.clear_and_free_semaphores(list(self.sems.allocated().values()))
        self.nc.all_engine_barrier()


def build_bass(n_layers=L):
    nc = bacc.Bacc(None)

    # ---- external I/O (per core; direction already baked by the host) ----
    x_in = nc.dram_tensor("x", [1, P, TOK], BF16, kind="ExternalInput")
    wih0_in = nc.dram_tensor("wih0", [I, G3], BF16, kind="ExternalInput")
    whh_in = nc.dram_tensor("whh", [n_layers, H, G3], WHH_DT, kind="ExternalInput")
    gxb_in = nc.dram_tensor("gxb", [n_layers, P, M3], F32, kind="ExternalInput")
    bhnb_in = nc.dram_tensor("bhnb", [n_layers, KH, P], BF16, kind="ExternalInput")
    bsel_in = nc.dram_tensor("bsel", [KH, KH * BC], BF16, kind="ExternalInput")
    ident_in = nc.dram_tensor("ident", [P, P], BF16, kind="ExternalInput")
    sel_in = nc.dram_tensor("sel", [P, 1], F32, kind="ExternalInput")
    out_d = nc.dram_tensor("out", [1, TOK], F32, kind="ExternalOutput")
    if n_layers > 1:
        wih_in = nc.dram_tensor(
            "wih", [n_layers - 1, 2 * H, G3], BF16, kind="ExternalInput"
        )
    fcw_in = nc.dram_tensor("fcw", [2 * H, 1], BF16, kind="ExternalInput")
    fcb_in = nc.dram_tensor("fcb", [1, 1], F32, kind="ExternalInput")

    # ---- internal DRAM: own-activation ping-pong [k, 128, tok] ----
    act_a = nc.dram_tensor("act_a", [KH, P, TOK], BF16)
    act_b = nc.dram_tensor("act_b", [KH, P, TOK], BF16)

    with ChunkedDrainTC(nc) as tc:
        with (
            tc.tile_pool(name="const", bufs=1) as cpool,
            tc.tile_pool(name="wpool", bufs=1) as wpool,
            tc.tile_pool(name="state", bufs=1) as state,
            tc.tile_pool(name="stage", bufs=2) as stage_pool,
            tc.tile_pool(name="pair", bufs=2) as pair_pool,
            tc.tile_pool(name="tmp", bufs=4) as tmp_pool,
            tc.tile_pool(name="ghR", bufs=1, space="PSUM") as ghR_pool,
            tc.tile_pool(name="ghZ", bufs=2, space="PSUM") as ghZ_pool,
            tc.tile_pool(name="ghB", bufs=2, space="PSUM") as ghB_pool,
            tc.tile_pool(name="gxps", bufs=3, space="PSUM") as gxps_pool,
            tc.tile_pool(name="dram", bufs=2, space="DRAM") as dram_pool,
        ):
            ident_sb = cpool.tile([P, P], BF16, tag="ident")
            nc.sync.dma_start(ident_sb[:], ident_in[:])
            sel_sb = cpool.tile([P, 1], F32, tag="sel")
            nc.sync.dma_start(sel_sb[:], sel_in[:])
            bsel_sb = cpool.tile([KH, KH * BC], BF16, tag="bsel")
            nc.sync.dma_start(bsel_sb[:], bsel_in[:])

            gathered = None  # previous layer's per-block AllGather results

            for layer in range(n_layers):
                ki = 1 if layer == 0 else 2 * KH
                act_own_prev = act_a if layer % 2 == 1 else act_b
                act_own = act_a if layer % 2 == 0 else act_b

                # ---- weights/biases for this layer ----
                whh_sb = wpool.tile([P, KH, G3], WHH_DT, tag="whh")
                nc.sync.dma_start(
                    whh_sb[:], whh_in[layer].rearrange("(ko p) m -> p ko m", p=P)
                )
                wih_sb = wpool.tile([P, ki, G3], BF16, tag="wih")
                src = (wih0_in[:] if layer == 0 else wih_in[layer - 1]).rearrange(
                    "(ko p) m -> p ko m", p=P
                )
                nc.sync.dma_start(wih_sb[:], src)
                gxb_sb = wpool.tile([P, M3], F32, tag="gxb")
                nc.sync.dma_start(gxb_sb[:], gxb_in[layer])
                bhn_sb = wpool.tile([KH, P], BF16, tag="bhn")
                nc.sync.dma_start(bhn_sb[:], bhnb_in[layer])

                # ---- per-layer state (step-major layouts) ----
                h_hist = state.tile([P, 2, 32, KH, BC], BF16, tag="hh")
                nc.vector.memset(h_hist[:, 1, 31], 0.0)  # t=0's h_prev slot
                gx_ring = state.tile([P, 2, 32, M3, BC], BF16, tag="gxr", name="gxr")

                if PER_BLOCK_CC:
                    rev_t = [dram_pool.tile([KH, P, BLKC], BF16, tag=f"rev{rb}",
                                            name=f"rev{rb}")
                             for rb in range(NBLK)]
                    gath_t = [dram_pool.tile([2, KH, P, BLKC], BF16,
                                             tag=f"gath{rb}", name=f"gath{rb}")
                              for rb in range(NBLK)]
                else:
                    rev_t = dram_pool.tile([NBLK, KH, P, BLKC], BF16, tag="rev")
                    gath_t = dram_pool.tile([2, NBLK, KH, P, BLKC], BF16, tag="gath")

                stage_sb = {}

                def emit_stage(tb):
                    """Stage block tb of this layer's input into SBUF."""
                    st = stage_pool.tile([P, ki, BLKC], BF16, tag="stage", name="st")
                    c0, c1 = tb * BLKC, (tb + 1) * BLKC
                    if layer == 0:
                        nc.sync.dma_start(st[:, 0, :], x_in[0, :, c0:c1])
                    else:
                        nc.sync.dma_start(
                            st[:, 0:KH, :],
                            act_own_prev[:, :, c0:c1].rearrange("k p c -> p k c"),
                        )
                        pr = pair_pool.tile([P, 2, KH, BLKC], BF16, tag="pr", name="pr")
                        for j in range(2):
                            gsrc = (gathered[tb][j] if PER_BLOCK_CC
                                    else gathered[j, tb])
                            nc.gpsimd.dma_start(
                                pr[:, j], gsrc.rearrange("k p c -> p k c")
                            )
                        dt_ = pair_pool.tile([P, KH, BLKC], BF16, tag="dsel", name="ds")
                        nc.vector.tensor_sub(dt_[:], pr[:, 0], pr[:, 1])
                        nc.vector.scalar_tensor_tensor(
                            st[:, KH : 2 * KH, :], dt_[:], sel_sb[:, 0:1], pr[:, 1],
                            OP.mult, OP.add,
                        )
                    stage_sb[tb] = st

                gx_open = {}  # (tb, m, half) -> psum tile

                def emit_gx_unit(tb, m, half, k):
                    """One k-chunk matmul of gx group (m, half) of block tb;
                    finishes with the PSUM->ring copy on the last chunk.
                    Returns the matmul instruction (for ordering deps)."""
                    st = stage_sb[tb]
                    key = (tb, m, half)
                    if k == 0:
                        gx_open[key] = gxps_pool.tile(
                            [P, HB], F32, tag="gxps", name="gxps"
                        )
                    ps = gx_open[key]
                    mm = nc.tensor.matmul(
                        ps[:],
                        wih_sb[:, k, m * P : (m + 1) * P],
                        st[:, k, half * HB : (half + 1) * HB],
                        start=(k == 0),
                        stop=(k == ki - 1),
                    )
                    if k == ki - 1:
                        par = tb % 2
                        # all gx shares the fp8 PSUM scale; descale folds into
                        # the sigmoid/tanh activation scale
                        sc = WSCALE
                        nc.scalar.activation(
                            gx_ring[:, par, half * 16 : (half + 1) * 16, m, :],
                            ps[:],
                            AF.Identity,
                            bias=gxb_sb[:, m : m + 1],
                            scale=sc,
                        )
                        del gx_open[key]
                    return mm

                # gx conveyor: units of (tb, half) are scheduled in the 16
                # global steps ending 2 steps before their first consumption
                # (CONVEYOR), or a whole block ahead (legacy).
                gx_due = {}
                for tb in range(NBLK):
                    for half in range(2):
                        units = [(tb, m, half, k) for m in range(M3)
                                 for k in range(ki)]
                        if CONVEYOR:
                            w0, wl = tb * 32 + half * 16 - 18, 16
                        else:
                            w0, wl = (tb - 1) * 32, 30
                        for i, u in enumerate(units):
                            s = w0 + (i * wl) // len(units)
                            gx_due.setdefault(max(s, -1), []).append(u)

                # ---- prologue: stage block 0 (and 1), block0/half0 gx ----
                emit_stage(0)
                for u in gx_due.get(-1, []):
                    emit_gx_unit(*u)

                def emit_flush(tb):
                    """Store finished hidden states of block tb to DRAM
                    (natural for own next-layer use, time-reversed for the
                    partner exchange), then exchange that reversed block."""
                    par = tb % 2
                    c0, c1 = tb * BLKC, (tb + 1) * BLKC
                    rb = NBLK - 1 - tb
                    rev_dst = rev_t[rb] if PER_BLOCK_CC else rev_t[rb]
                    for k in range(KH):
                        nc.gpsimd.dma_start(
                            act_own[k, :, c0:c1].rearrange("p (ts b) -> p ts b", ts=32),
                            h_hist[:, par, :, k, :],
                        )
                        dst = rev_dst[k].rearrange("p (ts b) -> p ts b", ts=32)
                        nc.gpsimd.dma_start(dst[:, ::-1, :], h_hist[:, par, :, k, :])
                    if PER_BLOCK_CC:
                        nc.gpsimd.collective_compute(
                            "AllGather",
                            OP.bypass,
                            replica_groups=[[c, c + 4] for c in range(4)],
                            ins=[rev_t[rb].opt()],
                            outs=[gath_t[rb].opt()],
                        )

                prev_tail = None  # last gx-unit mm of the previous step
                for tb in range(NBLK):
                    par = tb % 2
                    if tb < NBLK - 1:
                        emit_stage(tb + 1)
                    for ts in range(32):
                        t = tb * 32 + ts
                        prev = t - 1
                        slp, pap = prev % 32, (prev // 32) % 2

                        ghR = ghR_pool.tile([P, KH, BC], F32, tag="ghR", name="ghR")
                        ghZ = ghZ_pool.tile([P, KH, BC], F32, tag="ghZ", name="ghZ")
                        ghB = ghB_pool.tile([P, KH, BC], F32, tag="ghB", name="ghB")
                        h_prev = h_hist[:, pap, slp]

                        def sweep(gh, m0, id_slice):
                            # id_slice: pre-load gx into PSUM (also clears the
                            # bank); None (n path): first matmul clears instead
                            idm = None
                            if id_slice is not None:
                                idm = nc.tensor.matmul(
                                    gh[:].rearrange("p m b -> p (m b)"),
                                    ident_sb[:],
                                    gx_ring[:, par, ts, id_slice, :],
                                    start=True,
                                    stop=False,
                                )
                            for mm in range(KH):
                                m = m0 + mm
                                for k in range(KH):
                                    nc.tensor.matmul(
                                        gh[:, mm, :],
                                        whh_sb[:, k, m * P : (m + 1) * P],
                                        h_prev[:, k, :],
                                        start=(id_slice is None and mm == 0
                                               and k == 0),
                                        stop=(mm == KH - 1 and k == KH - 1
                                              and m0 != 8),
                                    )
                            return idm

                        # z gates first: sigmoid(z) / 1-z can start mid-sweep
                        idgx = sweep(ghZ, 4, slice(4, 8))
                        z_t = tmp_pool.tile([P, KH, BC], BF16, tag="zt", name="zt")
                        nc.scalar.activation(z_t[:], ghZ[:], AF.Sigmoid,
                                             scale=1.0 / WSCALE)
                        omz = tmp_pool.tile([P, KH, BC], BF16, tag="omz", name="oz")
                        nc.scalar.activation(omz[:], ghZ[:], AF.Sigmoid,
                                             scale=-1.0 / WSCALE)
                        zh = tmp_pool.tile([P, KH, BC], BF16, tag="zh", name="zh")
                        nc.vector.tensor_mul(zh[:], z_t[:], h_prev)
                        # r gates
                        sweep(ghR, 0, slice(0, 4))
                        r_t = tmp_pool.tile([P, KH, BC], BF16, tag="rt", name="rt")
                        nc.scalar.activation(r_t[:], ghR[:], AF.Sigmoid,
                                             scale=1.0 / WSCALE)
                        # n pre-activations (no gx pre-load: xn is added after
                        # the r-multiply)
                        sweep(ghB, 8, None)
                        bias_mm = nc.tensor.matmul(
                            ghB[:].rearrange("p m b -> p (m b)"),
                            bhn_sb[:],
                            bsel_sb[:],
                            start=False,
                            stop=True,
                        )
                        if ORDER_DEPS and prev_tail is not None:
                            _add_dep_helper(idgx.ins, prev_tail.ins, sync=False,
                                            reason="gx conveyor order")
                        prev_tail = None
                        for u in gx_due.get(t, []):
                            gmm = emit_gx_unit(*u)
                            if ORDER_DEPS:
                                _add_dep_helper(gmm.ins, bias_mm.ins, sync=False,
                                                reason="gx after rec sweep")
                                prev_tail = gmm

                        # n path (everything still x S; descale inside tanh)
                        tn = tmp_pool.tile([P, KH, BC], BF16, tag="tn", name="tn")
                        nc.vector.tensor_mul(tn[:], ghB[:], r_t[:])
                        np_ = tmp_pool.tile([P, KH, BC], BF16, tag="np", name="np")
                        nc.vector.tensor_add(np_[:], tn[:], gx_ring[:, par, ts, 8:12, :])
                        n_t = tmp_pool.tile([P, KH, BC], BF16, tag="nt", name="nt")
                        nc.scalar.activation(n_t[:], np_[:], AF.Tanh,
                                             scale=1.0 / WSCALE)
                        nm = tmp_pool.tile([P, KH, BC], BF16, tag="nm", name="nm")
                        nc.vector.tensor_mul(nm[:], n_t[:], omz[:])
                        nc.vector.tensor_add(h_hist[:, par, ts], nm[:], zh[:])
                    emit_flush(tb)

                if not PER_BLOCK_CC:
                    nc.gpsimd.collective_compute(
                        "AllGather",
                        OP.bypass,
                        replica_groups=[[c, c + 4] for c in range(4)],
                        ins=[rev_t.opt()],
                        outs=[gath_t.opt()],
                    )
                gathered = gath_t

            # ---- final FC + sigmoid ----
            act_own_prev = act_a if n_layers % 2 == 1 else act_b
            fcw_sb = wpool.tile([P, 2 * KH, 1], BF16, tag="fcw")
            nc.sync.dma_start(fcw_sb[:], fcw_in.rearrange("(ko p) n -> p ko n", p=P))
            fcb_sb = wpool.tile([1, 1], F32, tag="fcb")
            nc.sync.dma_start(fcb_sb[:], fcb_in[:])
            for tb in range(NBLK):
                c0, c1 = tb * BLKC, (tb + 1) * BLKC
                st = stage_pool.tile([P, 2 * KH, BLKC], BF16, tag="stage")
                nc.sync.dma_start(
                    st[:, 0:KH, :],
                    act_own_prev[:, :, c0:c1].rearrange("k p c -> p k c"),
                )
                pr = pair_pool.tile([P, 2, KH, BLKC], BF16, tag="pr")
                for j in range(2):
                    gsrc = gathered[tb][j] if PER_BLOCK_CC else gathered[j, tb]
                    nc.gpsimd.dma_start(
                        pr[:, j], gsrc.rearrange("k p c -> p k c")
                    )
                dt_ = pair_pool.tile([P, KH, BLKC], BF16, tag="dsel")
                nc.vector.tensor_sub(dt_[:], pr[:, 0], pr[:, 1])
                nc.vector.scalar_tensor_tensor(
                    st[:, KH : 2 * KH, :], dt_[:], sel_sb[:, 0:1], pr[:, 1],
                    OP.mult, OP.add,
                )
                for half in range(2):
                    ps = gxps_pool.tile([1, HB], F32, tag="gxps")
                    for k in range(2 * KH):
                        nc.tensor.matmul(
                            ps[:],
                            fcw_sb[:, k, :],
                            st[:, k, half * HB : (half + 1) * HB],
                            start=(k == 0),
                            stop=(k == 2 * KH - 1),
                        )
                    ob = tmp_pool.tile([1, HB], F32, tag="ob")
                    nc.scalar.activation(ob[:], ps[:], AF.Sigmoid,
                                         bias=fcb_sb[:, 0:1])
                    nc.sync.dma_start(
                        out_d[0:1, c0 + half * HB : c0 + (half + 1) * HB], ob[:]
                    )

    nc.finalize()
    return nc


def prep_inputs(input_seq, W_ih0, W_hh0, b_ih0, b_hh0, W_ih, W_hh, b_ih, b_hh,
                fc_w, fc_b, n_layers=L):
    """Host-side prep: per-core direction bake, transposes, bias folding."""
    bf = ml_dtypes.bfloat16
    whh_np = ml_dtypes.float8_e4m3fn if WHH_FP8 else bf

    x = np.asarray(input_seq)
    wih_all = [np.asarray(W_ih0)] + [np.asarray(W_ih)[l] for l in range(n_layers - 1)]
    whh_all = [np.asarray(W_hh0)] + [np.asarray(W_hh)[l] for l in range(n_layers - 1)]
    bih_all = [np.asarray(b_ih0)] + [np.asarray(b_ih)[l] for l in range(n_layers - 1)]
    bhh_all = [np.asarray(b_hh0)] + [np.asarray(b_hh)[l] for l in range(n_layers - 1)]

    ident = np.eye(P, dtype=bf)
    # bsel[k, m*BC+b] = 1 if k == m  (indicator rhs for the b_hn matmul)
    bsel = np.repeat(np.eye(KH, dtype=np.float32), BC, axis=1).astype(bf)
    in_maps = []
    for c in range(NCORES):
        d = 0 if c < 4 else 1
        shard = c % 4
        od = 1 - d

        whh = np.stack(
            [whh_all[l][d].T * WSCALE for l in range(n_layers)]
        ).astype(whh_np)
        wih0 = wih_all[0][d].T.astype(bf)  # [I, G3]
        wihs = []
        for l in range(1, n_layers):
            W = wih_all[l][d]  # [3H, 2H]; input feats [fwd H, bwd H]
            Wre = np.concatenate([W[:, d * H : (d + 1) * H],
                                  W[:, od * H : (od + 1) * H]], axis=1)
            wihs.append(Wre.T)  # [2H, G3]
        gxb = np.stack([bih_all[l][d].copy() for l in range(n_layers)])
        for l in range(n_layers):
            gxb[l][: 2 * H] += bhh_all[l][d][: 2 * H]
            gxb[l] *= WSCALE  # all gx shares the fp8 PSUM scale
        gxb = np.ascontiguousarray(
            np.transpose(gxb.reshape(n_layers, M3, P), (0, 2, 1))
        ).astype(np.float32)
        bhnb = np.stack(
            [bhh_all[l][d][2 * H :].reshape(KH, P) * WSCALE
             for l in range(n_layers)]
        ).astype(bf)

        fw = np.asarray(fc_w)[0]  # [2H]
        fcw = np.concatenate([fw[d * H : (d + 1) * H], fw[od * H : (od + 1) * H]])

        xc = x[:, shard * BC : (shard + 1) * BC, :]
        if d == 1:
            xc = xc[::-1]
        xc = np.ascontiguousarray(xc.reshape(TOK, I).T)[None]

        m = {
            "x": xc.astype(bf),
            "wih0": np.ascontiguousarray(wih0),
            "whh": np.ascontiguousarray(whh),
            "gxb": gxb,
            "bhnb": np.ascontiguousarray(bhnb),
            "ident": ident,
            "bsel": bsel,
            "sel": np.full((P, 1), float(d), np.float32),
            "fcw": np.ascontiguousarray(fcw[:, None]).astype(bf),
            "fcb": np.asarray(fc_b, dtype=np.float32).reshape(1, 1),
        }
        if n_layers > 1:
            m["wih"] = np.ascontiguousarray(np.stack(wihs)).astype(bf)
        in_maps.append(m)
    return in_maps


def assemble_output(results):
    """Forward cores 0-3 hold the output for batch shards 0-3."""
    outs = []
    for c in range(4):
        o = np.asarray(results[c]["out"]).reshape(T, BC)
        outs.append(o)
    return np.concatenate(outs, axis=1)[:, :, None].astype(np.float32)


def kernel(**inputs):
    nc = build_bass()
    in_maps = prep_inputs(**inputs)
    res = run_bass_kernel_spmd(nc, in_maps, list(range(NCORES)))
    return assemble_output(res.results)
